# revision 1
# baseline (speedup 1.0000x reference)
"""ASTGCN block kernel for 8 Trainium2 NeuronCores.

Pure data parallel: batch dim B=4096 sharded 512-per-core across the 8
cores; all params replicated. The per-core computation is expressed in
JAX and compiled for the NeuronCores through the PJRT backend (shard_map
over an 8-device mesh), so all compute runs on the trn2 devices.

Layout strategy: transpose x once to (b, n, t, f) and keep every large
tensor in a (..., t, channel) layout so that all heavy contractions are
last-axis matmuls. This avoids the compiler-inserted NKI transpose
kernels that dominated the earlier NCHW-conv formulation (the two convs
are rewritten as shifted-slice matmuls; x_tat is eliminated
algebraically, its two uses contract to small (b,n,t)-sized ops).
"""

import numpy as np

B, N, F_IN, T = 4096, 38, 64, 5
K, C_CHEB, C_TIME = 3, 64, 64
EPS = 1e-5
NCORES = 8

_cache = {}


def _get_compiled():
    if "fn" in _cache:
        return _cache["fn"]
    import jax
    import jax.numpy as jnp
    from jax.sharding import Mesh, PartitionSpec as P
    from jax.experimental.shard_map import shard_map

    devs = jax.devices()
    nd = NCORES
    while nd > 1 and (len(devs) < nd or B % nd != 0):
        nd //= 2
    devs = devs[:nd]
    mesh = Mesh(np.array(devs), ("x",))

    def block(x, cheb, U1, U2, U3, b_e, V_e, W1, W2, W3, b_s, V_s,
              Theta, W_time, b_time, W_res, b_res, gamma, beta):
        b = x.shape[0]
        # one layout change up front: (b,n,f,t) -> (b,n,t,f)
        xt = jnp.transpose(x, (0, 1, 3, 2))

        # ---- temporal attention ----
        lhs1 = jnp.einsum('bntf,n->btf', xt, U1)
        lhs = jnp.einsum('btf,fn->btn', lhs1, U2)          # (b,T,N)
        rhs = jnp.einsum('bntf,f->bnt', xt, U3)            # (b,N,T)
        prod = jnp.einsum('btn,bns->bts', lhs, rhs)        # (b,T,T)
        E = jnp.einsum('btj,ij->bti', jax.nn.sigmoid(prod + b_e), V_e)
        t_at = jax.nn.softmax(E, axis=1)                   # (b,T,T)

        # ---- spatial attention (x_tat eliminated algebraically) ----
        w1t = jnp.einsum('bts,s->bt', t_at, W1)            # (b,T)
        sl1 = jnp.einsum('bntf,bt->bnf', xt, w1t)          # (b,N,F)
        sl = jnp.einsum('bnf,ft->bnt', sl1, W2)            # (b,N,T)
        sr = jnp.einsum('bmt,bts->bms', rhs, t_at)         # (b,N,T)
        sp = jnp.einsum('bnt,bmt->bnm', sl, sr)            # (b,N,N)
        S = jnp.einsum('nk,bkm->bnm', V_s, jax.nn.sigmoid(sp + b_s))
        s_at = jax.nn.softmax(S, axis=1)                   # (b,N,N)

        # ---- K-order Chebyshev conv with spatial attention ----
        # gcn[b,n,t,o] = relu(sum_k sum_m cheb[k,m,n]*s_at[b,m,n]
        #                      * sum_f xt[b,m,t,f] Theta[k,f,o])
        tk_at = cheb[None] * s_at[:, None]                 # (b,K,N,N)
        # P: one big matmul (b*N*T, F) @ (F, K*C)
        Pf = jnp.einsum('bmtf,fz->bmtz', xt,
                        jnp.transpose(Theta, (1, 0, 2)).reshape(F_IN, K * C_CHEB))
        Pf = Pf.reshape(b, N, T, K, C_CHEB)
        gcn = jnp.zeros((b, N, T * C_CHEB), x.dtype)
        for k in range(K):
            Pk = Pf[:, :, :, k, :].reshape(b, N, T * C_CHEB)
            gcn = gcn + jnp.einsum('bmn,bmz->bnz', tk_at[:, k], Pk)
        gcn = jax.nn.relu(gcn).reshape(b, N, T, C_CHEB)    # (b,N,T,C)

        # ---- temporal conv (1,3) pad (0,1): shifted-slice matmuls ----
        gp = jnp.pad(gcn, ((0, 0), (0, 0), (1, 1), (0, 0)))
        tco = b_time[None, None, None, :]
        for w in range(3):
            tco = tco + jnp.einsum('bnti,ci->bntc',
                                   gp[:, :, w:w + T, :], W_time[:, :, 0, w])

        # ---- 1x1 residual conv ----
        res = jnp.einsum('bntf,cf->bntc', xt, W_res[:, :, 0, 0]) \
            + b_res[None, None, None, :]

        # ---- residual add, relu, layernorm over channel (last axis) ----
        h = jax.nn.relu(res + tco)                         # (b,N,T,C)
        mu = jnp.mean(h, axis=-1, keepdims=True)
        var = jnp.mean(jnp.square(h - mu), axis=-1, keepdims=True)
        ln = gamma * (h - mu) * jax.lax.rsqrt(var + EPS) + beta
        return jnp.transpose(ln, (0, 1, 3, 2))             # (b,N,C,T)

    def block_bf16mm(*args):
        # Single-pass bf16 matmuls (fp32 dots otherwise expand to multi-pass
        # LOW_HIGH on the PE); fp32 accumulate keeps error ~1e-3.
        with jax.default_matmul_precision("bfloat16"):
            return block(*args)

    pspec_x = P("x")          # shard batch dim
    pspec_rep = P()           # replicated params
    in_specs = (pspec_x,) + (pspec_rep,) * 18
    fn = jax.jit(
        shard_map(block_bf16mm, mesh=mesh, in_specs=in_specs,
                  out_specs=pspec_x)
    )
    _cache["fn"] = fn
    return fn


def kernel(x, cheb, U1, U2, U3, b_e, V_e, W1, W2, W3, b_s, V_s,
           Theta, W_time, b_time, W_res, b_res, gamma, beta):
    import jax.numpy as jnp

    fn = _get_compiled()
    args = [x, cheb, U1, U2, U3, b_e, V_e, W1, W2, W3, b_s, V_s,
            Theta, W_time, b_time, W_res, b_res, gamma, beta]
    args = [jnp.asarray(np.asarray(a), jnp.float32) for a in args]
    out = fn(*args)
    return np.asarray(out, dtype=np.float32)



# revision 5
# speedup vs baseline: 1.0323x; 1.0323x over previous
"""ASTGCN block kernel for 8 Trainium2 NeuronCores.

Pure data parallel: batch dim B=4096 sharded 512-per-core across the 8
cores; all params replicated. The per-core computation is expressed in
JAX and compiled for the NeuronCores through the PJRT backend (shard_map
over an 8-device mesh), so all compute runs on the trn2 devices.

Layout strategy: transpose x once to (b, n, t, f) and keep every large
tensor in a (..., t, channel) layout so that all heavy contractions are
last-axis matmuls (avoids compiler-inserted NKI transpose kernels).

v2 changes vs the first working version:
- Chebyshev conv runs graph-contraction FIRST (reference order), then
  the shared Theta matmul per k. This avoids materializing the
  (b, N, T, K*C) = 143MB/core Pf intermediate; the per-k Z tensor is
  only (b, N, T, F) = 48MB (24MB in bf16). The problem is memory-bound,
  so intermediate HBM traffic dominates.
- x and all large intermediates are cast to bf16 (fp32 accumulation in
  matmuls via preferred_element_type); halves HBM traffic. Small
  attention tensors and the LN epilogue stay fp32.
"""

import numpy as np

B, N, F_IN, T = 4096, 38, 64, 5
K, C_CHEB, C_TIME = 3, 64, 64
EPS = 1e-5
NCORES = 8

_cache = {}


def _get_compiled():
    if "fn" in _cache:
        return _cache["fn"]
    import jax
    import jax.numpy as jnp
    from jax.sharding import Mesh, PartitionSpec as P
    from jax.experimental.shard_map import shard_map

    devs = jax.devices()
    nd = NCORES
    while nd > 1 and (len(devs) < nd or B % nd != 0):
        nd //= 2
    devs = devs[:nd]
    mesh = Mesh(np.array(devs), ("x",))

    def block(x, cheb, U1, U2, U3, b_e, V_e, W1, W2, W3, b_s, V_s,
              Theta, W_time, b_time, W_res, b_res, gamma, beta):
        b = x.shape[0]
        bf = jnp.bfloat16
        # one layout change up front: (b,n,f,t) -> (b,n,t,f), in bf16
        xt = jnp.transpose(x.astype(bf), (0, 1, 3, 2))

        # ---- temporal attention ----
        lhs1 = jnp.einsum('bntf,n->btf', xt, U1.astype(bf))
        lhs = jnp.einsum('btf,fn->btn', lhs1, U2.astype(bf))   # (b,T,N)
        rhs = jnp.einsum('bntf,f->bnt', xt, U3.astype(bf))     # (b,N,T)
        prod = jnp.einsum('btn,bns->bts', lhs, rhs,
                          preferred_element_type=jnp.float32)  # (b,T,T)
        E = jnp.einsum('btj,ij->bti', jax.nn.sigmoid(prod + b_e), V_e)
        t_at = jax.nn.softmax(E, axis=1)                       # (b,T,T)

        # ---- spatial attention (x_tat eliminated algebraically) ----
        w1t = jnp.einsum('bts,s->bt', t_at, W1)                # (b,T)
        sl1 = jnp.einsum('bntf,bt->bnf', xt, w1t.astype(bf))   # (b,N,F)
        sl = jnp.einsum('bnf,ft->bnt', sl1, W2.astype(bf))     # (b,N,T)
        sr = jnp.einsum('bmt,bts->bms', rhs.astype(jnp.float32), t_at)
        sp = jnp.einsum('bnt,bmt->bnm', sl.astype(jnp.float32), sr)
        S = jnp.einsum('nk,bkm->bnm', V_s, jax.nn.sigmoid(sp + b_s))
        s_at = jax.nn.softmax(S, axis=1)                       # (b,N,N)

        # ---- K-order Chebyshev conv, graph contraction first ----
        # gcn[b,n,t,o] = relu(sum_k (sum_m tk_at[b,k,m,n] xt[b,m,t,f]) @ Theta[k])
        s_at16 = s_at.astype(bf)
        cheb16 = cheb.astype(bf)
        gcn = jnp.zeros((b, N, T, C_CHEB), jnp.float32)
        for k in range(K):
            tk = cheb16[k][None] * s_at16                      # (b,m,n)
            Zk = jnp.einsum('bmn,bmtf->bntf', tk, xt)          # (b,N,T,F) bf16
            gcn = gcn + jnp.einsum('bntf,fo->bnto', Zk, Theta[k].astype(bf),
                                   preferred_element_type=jnp.float32)
        gcn = jax.nn.relu(gcn).astype(bf)                      # (b,N,T,C)

        # ---- temporal conv (1,3) pad (0,1): shifted-slice matmuls ----
        gp = jnp.pad(gcn, ((0, 0), (0, 0), (1, 1), (0, 0)))
        tco = b_time[None, None, None, :]
        for w in range(3):
            tco = tco + jnp.einsum('bnti,ci->bntc',
                                   gp[:, :, w:w + T, :], W_time[:, :, 0, w].astype(bf),
                                   preferred_element_type=jnp.float32)

        # ---- 1x1 residual conv ----
        res = jnp.einsum('bntf,cf->bntc', xt, W_res[:, :, 0, 0].astype(bf),
                         preferred_element_type=jnp.float32) \
            + b_res[None, None, None, :]

        # ---- residual add, relu, layernorm over channel (last axis) ----
        h = jax.nn.relu(res + tco)                             # (b,N,T,C) f32
        mu = jnp.mean(h, axis=-1, keepdims=True)
        var = jnp.mean(jnp.square(h - mu), axis=-1, keepdims=True)
        ln = gamma * (h - mu) * jax.lax.rsqrt(var + EPS) + beta
        return jnp.transpose(ln, (0, 1, 3, 2))                 # (b,N,C,T)

    _cache["body"] = block

    def block_bf16mm(*args):
        with jax.default_matmul_precision("bfloat16"):
            return block(*args)

    pspec_x = P("x")          # shard batch dim
    pspec_rep = P()           # replicated params
    in_specs = (pspec_x,) + (pspec_rep,) * 18
    fn = jax.jit(
        shard_map(block_bf16mm, mesh=mesh, in_specs=in_specs,
                  out_specs=pspec_x)
    )
    _cache["fn"] = fn
    return fn


def _get_compiled_loop(n_iter):
    """Same block chained n_iter times with a serial data dependency.

    Used only for timing: the marginal time per extra iteration is the
    true device execution time, free of the fixed per-launch RPC cost of
    the axon tunnel. A tiny feedback term (x + 1e-6*out) makes each
    iteration depend on the previous one so XLA cannot CSE them.
    """
    key = ("loop", n_iter)
    if key in _cache:
        return _cache[key]
    import jax
    import jax.numpy as jnp
    from jax.sharding import Mesh, PartitionSpec as P
    from jax.experimental.shard_map import shard_map

    devs = jax.devices()
    nd = NCORES
    while nd > 1 and (len(devs) < nd or B % nd != 0):
        nd //= 2
    devs = devs[:nd]
    mesh = Mesh(np.array(devs), ("x",))
    body = _cache["body"]

    def looped(*args):
        x = args[0]
        rest = args[1:]
        out = None
        with jax.default_matmul_precision("bfloat16"):
            for _ in range(n_iter):
                out = body(x, *rest)
                # out is (b,N,C,T) with C == F_IN, same shape as x
                x = x + 1e-6 * out
        return out

    pspec_x = P("x")
    in_specs = (pspec_x,) + (P(),) * 18
    fn = jax.jit(shard_map(looped, mesh=mesh, in_specs=in_specs,
                           out_specs=pspec_x))
    _cache[key] = fn
    return fn


def kernel(x, cheb, U1, U2, U3, b_e, V_e, W1, W2, W3, b_s, V_s,
           Theta, W_time, b_time, W_res, b_res, gamma, beta):
    import jax.numpy as jnp

    fn = _get_compiled()
    args = [x, cheb, U1, U2, U3, b_e, V_e, W1, W2, W3, b_s, V_s,
            Theta, W_time, b_time, W_res, b_res, gamma, beta]
    args = [jnp.asarray(np.asarray(a), jnp.float32) for a in args]
    out = fn(*args)
    return np.asarray(out, dtype=np.float32)


# revision 7
# speedup vs baseline: 115981340.0000x; 112348794.0000x over previous
"""ASTGCN block kernel for 8 Trainium2 NeuronCores.

Pure data parallel: batch dim B=4096 sharded 512-per-core across the 8
cores; all params replicated. The per-core computation is expressed in
JAX and compiled for the NeuronCores through the PJRT backend (shard_map
over an 8-device mesh), so all compute runs on the trn2 devices.

Layout strategy: transpose x once to (b, n, t, f) and keep every large
tensor in a (..., t, channel) layout so that all heavy contractions are
last-axis matmuls (avoids compiler-inserted NKI transpose kernels).

v2 changes vs the first working version:
- Chebyshev conv runs graph-contraction FIRST (reference order), then
  the shared Theta matmul per k. This avoids materializing the
  (b, N, T, K*C) = 143MB/core Pf intermediate; the per-k Z tensor is
  only (b, N, T, F) = 48MB (24MB in bf16). The problem is memory-bound,
  so intermediate HBM traffic dominates.
- x and all large intermediates are cast to bf16 (fp32 accumulation in
  matmuls via preferred_element_type); halves HBM traffic. Small
  attention tensors and the LN epilogue stay fp32.
"""

import numpy as np

B, N, F_IN, T = 4096, 38, 64, 5
K, C_CHEB, C_TIME = 3, 64, 64
EPS = 1e-5
NCORES = 8

_cache = {}


def _get_compiled():
    if "fn" in _cache:
        return _cache["fn"]
    import jax
    import jax.numpy as jnp
    from jax.sharding import Mesh, PartitionSpec as P
    from jax.experimental.shard_map import shard_map

    devs = jax.devices()
    nd = NCORES
    while nd > 1 and (len(devs) < nd or B % nd != 0):
        nd //= 2
    devs = devs[:nd]
    mesh = Mesh(np.array(devs), ("x",))

    def block(x, cheb, U1, U2, U3, b_e, V_e, W1, W2, W3, b_s, V_s,
              Theta, W_time, b_time, W_res, b_res, gamma, beta):
        b = x.shape[0]
        bf = jnp.bfloat16
        # one layout change up front: (b,n,f,t) -> (b,n,t,f), in bf16
        xt = jnp.transpose(x.astype(bf), (0, 1, 3, 2))

        # ---- temporal attention ----
        lhs1 = jnp.einsum('bntf,n->btf', xt, U1.astype(bf))
        lhs = jnp.einsum('btf,fn->btn', lhs1, U2.astype(bf))   # (b,T,N)
        rhs = jnp.einsum('bntf,f->bnt', xt, U3.astype(bf))     # (b,N,T)
        prod = jnp.einsum('btn,bns->bts', lhs, rhs,
                          preferred_element_type=jnp.float32)  # (b,T,T)
        E = jnp.einsum('btj,ij->bti', jax.nn.sigmoid(prod + b_e), V_e)
        t_at = jax.nn.softmax(E, axis=1)                       # (b,T,T)

        # ---- spatial attention (x_tat eliminated algebraically) ----
        w1t = jnp.einsum('bts,s->bt', t_at, W1)                # (b,T)
        sl1 = jnp.einsum('bntf,bt->bnf', xt, w1t.astype(bf))   # (b,N,F)
        sl = jnp.einsum('bnf,ft->bnt', sl1, W2.astype(bf))     # (b,N,T)
        sr = jnp.einsum('bmt,bts->bms', rhs.astype(jnp.float32), t_at)
        sp = jnp.einsum('bnt,bmt->bnm', sl, sr.astype(bf),
                        preferred_element_type=jnp.float32)
        S = jnp.einsum('nk,bkm->bnm', V_s, jax.nn.sigmoid(sp + b_s))
        s_at = jax.nn.softmax(S, axis=1)                       # (b,N,N)

        # ---- K-order Chebyshev conv, graph contraction first ----
        # gcn[b,n,t,o] = relu(sum_k (sum_m tk_at[b,k,m,n] xt[b,m,t,f]) @ Theta[k])
        s_at16 = s_at.astype(bf)
        cheb16 = cheb.astype(bf)
        gcn = jnp.zeros((b, N, T, C_CHEB), jnp.float32)
        for k in range(K):
            tk = cheb16[k][None] * s_at16                      # (b,m,n)
            Zk = jnp.einsum('bmn,bmtf->bntf', tk, xt)          # (b,N,T,F) bf16
            gcn = gcn + jnp.einsum('bntf,fo->bnto', Zk, Theta[k].astype(bf),
                                   preferred_element_type=jnp.float32)
        gcn = jax.nn.relu(gcn).astype(bf)                      # (b,N,T,C)

        # ---- temporal conv (1,3) pad (0,1): shifted-slice matmuls.
        # Outputs are produced directly in the final (b,N,C,T) layout so
        # the epilogue needs no 50MB transpose.
        gp = jnp.pad(gcn, ((0, 0), (0, 0), (1, 1), (0, 0)))
        tco = b_time[None, None, :, None]
        for w in range(3):
            tco = tco + jnp.einsum('bnti,ci->bnct',
                                   gp[:, :, w:w + T, :], W_time[:, :, 0, w].astype(bf),
                                   preferred_element_type=jnp.float32)

        # ---- 1x1 residual conv ----
        res = jnp.einsum('bntf,cf->bnct', xt, W_res[:, :, 0, 0].astype(bf),
                         preferred_element_type=jnp.float32) \
            + b_res[None, None, :, None]

        # ---- residual add, relu, layernorm over channel (axis 2) ----
        h = jax.nn.relu(res + tco)                             # (b,N,C,T) f32
        mu = jnp.mean(h, axis=2, keepdims=True)
        var = jnp.mean(jnp.square(h - mu), axis=2, keepdims=True)
        ln = gamma[None, None, :, None] * (h - mu) * jax.lax.rsqrt(var + EPS) \
            + beta[None, None, :, None]
        return ln                                              # (b,N,C,T)

    _cache["body"] = block

    def block_bf16mm(*args):
        with jax.default_matmul_precision("bfloat16"):
            return block(*args)

    pspec_x = P("x")          # shard batch dim
    pspec_rep = P()           # replicated params
    in_specs = (pspec_x,) + (pspec_rep,) * 18
    fn = jax.jit(
        shard_map(block_bf16mm, mesh=mesh, in_specs=in_specs,
                  out_specs=pspec_x)
    )
    _cache["fn"] = fn
    return fn


def _get_compiled_loop(n_iter):
    """Same block chained n_iter times with a serial data dependency.

    Used only for timing: the marginal time per extra iteration is the
    true device execution time, free of the fixed per-launch RPC cost of
    the axon tunnel. A tiny feedback term (x + 1e-6*out) makes each
    iteration depend on the previous one so XLA cannot CSE them.
    """
    key = ("loop", n_iter)
    if key in _cache:
        return _cache[key]
    import jax
    import jax.numpy as jnp
    from jax.sharding import Mesh, PartitionSpec as P
    from jax.experimental.shard_map import shard_map

    devs = jax.devices()
    nd = NCORES
    while nd > 1 and (len(devs) < nd or B % nd != 0):
        nd //= 2
    devs = devs[:nd]
    mesh = Mesh(np.array(devs), ("x",))
    body = _cache["body"]

    def looped(*args):
        x = args[0]
        rest = args[1:]
        out = None
        with jax.default_matmul_precision("bfloat16"):
            for _ in range(n_iter):
                out = body(x, *rest)
                # out is (b,N,C,T) with C == F_IN, same shape as x
                x = x + 1e-6 * out
        return out

    pspec_x = P("x")
    in_specs = (pspec_x,) + (P(),) * 18
    fn = jax.jit(shard_map(looped, mesh=mesh, in_specs=in_specs,
                           out_specs=pspec_x))
    _cache[key] = fn
    return fn


def kernel(x, cheb, U1, U2, U3, b_e, V_e, W1, W2, W3, b_s, V_s,
           Theta, W_time, b_time, W_res, b_res, gamma, beta):
    import jax.numpy as jnp

    fn = _get_compiled()
    args = [x, cheb, U1, U2, U3, b_e, V_e, W1, W2, W3, b_s, V_s,
            Theta, W_time, b_time, W_res, b_res, gamma, beta]
    args = [jnp.asarray(np.asarray(a), jnp.float32) for a in args]
    out = fn(*args)
    return np.asarray(out, dtype=np.float32)
